# revision 3
# baseline (speedup 1.0000x reference)
import sys

sys.path.insert(0, "/opt/trn_rl_repo")
import numpy as np

# nn_BisineNetwork: out[n,c] = sum_k a[c,k] * sin(x@w1[c,k]+b1[c,k]) * sin(x@w2[c,k]+b2[c,k])
# Shapes (hardcoded): x (16384, 256) f32, params (1000, 2060) f32 -> out (16384, 1000) f32.
#
# Sharding: data-parallel over batch N across 8 cores (N_shard = 2048); params
# replicated. Per-core layout is [ck, n] (c,k merged -> 4000, padded to 4096).
# W is pre-scaled by 1/2pi on host so u arrives in "turns":
#   u1 = W1blk.T @ Xshard          (PE fp16, contraction d=256 in 2 chunks, psum f32)
#   m1 = wrap(u1 + b1') in [-.5,.5] (custom DVE op: magic-number round, 1 pass)
#   q1 = sin(2pi * m1)              (ACT Sin via free scale, fp16 out)
#   prod = q1 * q2                  (GPSIMD, fp32r out)
#   outT[cblk] += A_j.T @ prod      (PE fp32r, reduction over k with a-coeffs)
# Host: transpose/pad/scale/cast prep of x and params; final transpose of outT.

D = 256
C = 1000
K = 4
CK = C * K          # 4000
CKP = 4096          # padded
NCORES = 8
N = 16384
NS = N // NCORES    # 2048 per core
NH = 1024           # n-span per step (2 psum banks)
TWO_PI = float(2 * np.pi)
MAGIC = 12582912.0  # 1.5 * 2**23: fp32 RNE round-to-int trick
_CACHE = {}


def _register_wrap_op():
    """out = y - round(y) with y = in0 + s0 (per-partition bias), via the
    fp32 magic-number trick: k = (y + MAGIC) - MAGIC. Exact for |y| < 2^21."""
    import re

    from concourse import dve_ops as DV
    from concourse.dve_spec import C0, C1, Spec, Src0

    for o in DV.OPS:
        if o.name == "BISINE_WRAP":
            return o

    def _ref(in0, in1, s0, s1, imm2):
        y = (np.asarray(in0, np.float32) + np.asarray(s0, np.float32)).astype(
            np.float32
        )
        t = (y + np.float32(s1)).astype(np.float32)
        k = (t - np.float32(s1)).astype(np.float32)
        return (y - k).astype(np.float32)

    y = Src0 + C0
    k = (y + C1) - C1
    op = DV.DveOp("BISINE_WRAP", Spec(body=y - k, reference=_ref), subdim=False, uops_sha={})
    DV.OPS.append(op)
    DV.CUSTOM_DVE_SPECS[op.name] = op.spec
    DV._SUB_OPCODE_FOR_NAME[op.name] = DV._CUSTOM_DVE_ROW_BASE + len(DV.OPS) - 1
    for ver in ("v3", "v4"):
        try:
            op.compile(ver)
        except ValueError as e:
            m = re.findall(r'="([0-9a-f]+)"', str(e))
            assert m, e
            op.uops_sha[ver] = m[-1]
            op.compile(ver)
    return op


def _build_nc():
    import concourse.bacc as bacc
    import concourse.mybir as mybir
    import concourse.tile as tile

    SIN = mybir.ActivationFunctionType.Sin
    ALU = mybir.AluOpType
    BF16 = mybir.dt.bfloat16
    F16 = mybir.dt.float16
    F32 = mybir.dt.float32
    F32R = mybir.dt.float32r

    wrap_op = _register_wrap_op()
    nc = bacc.Bacc("TRN2", target_bir_lowering=False, debug=False)

    xt_d = nc.dram_tensor("xt", [D, NS], F16, kind="ExternalInput")
    w1_d = nc.dram_tensor("w1t", [D, CKP], F16, kind="ExternalInput")
    w2_d = nc.dram_tensor("w2t", [D, CKP], F16, kind="ExternalInput")
    a_d = nc.dram_tensor("acoef", [CKP, 128], F32R, kind="ExternalInput")
    b1_d = nc.dram_tensor("b1v", [CKP, 1], F32, kind="ExternalInput")
    b2_d = nc.dram_tensor("b2v", [CKP, 1], F32, kind="ExternalInput")
    out_d = nc.dram_tensor("outT", [CKP // 4, NS], F32, kind="ExternalOutput")

    NJ = CKP // 128  # 32 ck-blocks
    NCB = CKP // 512  # 8 c-blocks (128 c each)

    with tile.TileContext(nc) as tc:
        with (
            tc.tile_pool(name="const", bufs=1) as cp,
            tc.tile_pool(name="work", bufs=2) as wp,
            tc.tile_pool(name="u1p", bufs=1, space="PSUM") as u1p,
            tc.tile_pool(name="u2p", bufs=1, space="PSUM") as u2p,
            tc.tile_pool(name="op", bufs=2, space="PSUM") as op,
        ):
            xt = cp.tile([128, 2, NS], F16, tag="xt")
            w1t = cp.tile([128, 2, CKP], F16, tag="w1t")
            w2t = cp.tile([128, 2, CKP], F16, tag="w2t")
            at = cp.tile([128, NJ, 128], F32R, tag="at")
            b1c = cp.tile([128, NJ], F32, tag="b1c")
            b2c = cp.tile([128, NJ], F32, tag="b2c")
            zero = cp.tile([128, 1], F32, tag="zero")

            nc.sync.dma_start(xt[:], xt_d.ap().rearrange("(c p) n -> p c n", p=128))
            nc.sync.dma_start(w1t[:], w1_d.ap().rearrange("(c p) k -> p c k", p=128))
            nc.sync.dma_start(w2t[:], w2_d.ap().rearrange("(c p) k -> p c k", p=128))
            nc.sync.dma_start(at[:], a_d.ap().rearrange("(j p) m -> p j m", p=128))
            nc.sync.dma_start(b1c[:], b1_d.ap().rearrange("(j p) o -> p (j o)", p=128))
            nc.sync.dma_start(b2c[:], b2_d.ap().rearrange("(j p) o -> p (j o)", p=128))
            nc.gpsimd.memset(zero[:], 0.0)
            # sin argument = SCALE*m with |m| <= 0.5; SCALE one ulp under 2pi
            # keeps it strictly inside the ACT Sin [-pi, pi] domain.
            SCALE = float(np.nextafter(np.float32(TWO_PI), np.float32(0.0)))

            for cb in range(NCB):
                for nh in range(NS // NH):
                    o_ps = op.tile([128, NH], F32, tag="o_ps")
                    for jj in range(4):
                        j = 4 * cb + jj
                        u1 = u1p.tile([128, NH], F32, tag="u1")
                        u2 = u2p.tile([128, NH], F32, tag="u2")
                        for h in range(NH // 512):
                            ncol = nh * NH + h * 512
                            rx0 = xt[:, 0, ncol : ncol + 512]
                            rx1 = xt[:, 1, ncol : ncol + 512]
                            c0, c1 = h * 512, (h + 1) * 512
                            jc = slice(128 * j, 128 * (j + 1))
                            nc.tensor.matmul(
                                u1[:, c0:c1], w1t[:, 0, jc], rx0, start=True, stop=False
                            )
                            nc.tensor.matmul(
                                u1[:, c0:c1], w1t[:, 1, jc], rx1, start=False, stop=True
                            )
                            nc.tensor.matmul(
                                u2[:, c0:c1], w2t[:, 0, jc], rx0, start=True, stop=False
                            )
                            nc.tensor.matmul(
                                u2[:, c0:c1], w2t[:, 1, jc], rx1, start=False, stop=True
                            )
                        m1 = wp.tile([128, NH], F32, tag="m1")
                        m2 = wp.tile([128, NH], F32, tag="m2")
                        q1 = wp.tile([128, NH], F16, tag="q1")
                        q2 = wp.tile([128, NH], F16, tag="q2")
                        prod = wp.tile([128, NH], F32R, tag="prod")
                        nc.vector._custom_dve(
                            wrap_op, out=m1[:], in0=u1[:], s0=b1c[:, j : j + 1], s1=MAGIC
                        )
                        nc.vector._custom_dve(
                            wrap_op, out=m2[:], in0=u2[:], s0=b2c[:, j : j + 1], s1=MAGIC
                        )
                        nc.scalar.activation(q1[:], m1[:], SIN, bias=zero[:], scale=SCALE)
                        nc.scalar.activation(q2[:], m2[:], SIN, bias=zero[:], scale=SCALE)
                        nc.gpsimd.tensor_mul(prod[:], q1[:], q2[:])
                        for h in range(NH // 512):
                            c0, c1 = h * 512, (h + 1) * 512
                            nc.tensor.matmul(
                                o_ps[:, c0:c1],
                                at[:, j, :],
                                prod[:, c0:c1],
                                start=(jj == 0),
                                stop=(jj == 3),
                            )
                    o_sb = wp.tile([128, NH], F32, tag="o_sb")
                    nc.vector.tensor_copy(o_sb[:], o_ps[:])
                    nc.sync.dma_start(
                        out_d.ap()[128 * cb : 128 * (cb + 1), nh * NH : (nh + 1) * NH],
                        o_sb[:],
                    )

    nc.compile()
    return nc


def _prep(x, params):
    p = np.asarray(params, dtype=np.float32).reshape(C, K, 2 * D + 3)
    a = np.ascontiguousarray(p[:, :, 0]).reshape(CK)
    w1 = np.ascontiguousarray(p[:, :, 1 : 1 + D]).reshape(CK, D)
    b1 = np.ascontiguousarray(p[:, :, 1 + D]).reshape(CK)
    w2 = np.ascontiguousarray(p[:, :, 2 + D : 2 + 2 * D]).reshape(CK, D)
    b2 = np.ascontiguousarray(p[:, :, 2 + 2 * D]).reshape(CK)

    w1p = np.zeros((CKP, D), np.float32)
    w2p = np.zeros((CKP, D), np.float32)
    w1p[:CK] = w1
    w2p[:CK] = w2
    inv2pi = np.float32(1.0 / TWO_PI)
    w1t = np.ascontiguousarray(w1p.T * inv2pi).astype(np.float16)
    w2t = np.ascontiguousarray(w2p.T * inv2pi).astype(np.float16)

    b1v = np.zeros((CKP, 1), np.float32)
    b2v = np.zeros((CKP, 1), np.float32)
    b1v[:CK, 0] = b1 * inv2pi
    b2v[:CK, 0] = b2 * inv2pi

    ap = np.zeros(CKP, np.float32)
    ap[:CK] = a
    # acoef[j*128+p, m] = ap[j*128+p] iff m == 32*(j%4) + p//4
    jj = np.arange(CKP) // 128
    pp = np.arange(CKP) % 128
    mcol = 32 * (jj % 4) + pp // 4
    acoef = np.zeros((CKP, 128), np.float32)
    acoef[np.arange(CKP), mcol] = ap

    xt = np.ascontiguousarray(np.asarray(x, dtype=np.float32).T).astype(np.float16)  # [D, N]
    return xt, w1t, w2t, acoef, b1v, b2v


def kernel(x, params):
    from concourse import bass_utils

    if "nc" not in _CACHE:
        _CACHE["nc"] = _build_nc()
    nc = _CACHE["nc"]

    xt, w1t, w2t, acoef, b1v, b2v = _prep(x, params)
    in_maps = []
    for cid in range(NCORES):
        in_maps.append(
            {
                "xt": np.ascontiguousarray(xt[:, cid * NS : (cid + 1) * NS]),
                "w1t": w1t,
                "w2t": w2t,
                "acoef": acoef,
                "b1v": b1v,
                "b2v": b2v,
            }
        )
    res = bass_utils.run_bass_kernel_spmd(nc, in_maps, core_ids=list(range(NCORES)))
    outs = [res.results[c]["outT"] for c in range(NCORES)]
    out_t = np.concatenate(outs, axis=1)  # [1024, 16384]
    return np.ascontiguousarray(out_t[:C].T)


# revision 5
# speedup vs baseline: 1.0658x; 1.0658x over previous
import sys

sys.path.insert(0, "/opt/trn_rl_repo")
import numpy as np

# nn_BisineNetwork: out[n,c] = sum_k a[c,k] * sin(x@w1[c,k]+b1[c,k]) * sin(x@w2[c,k]+b2[c,k])
# Shapes (hardcoded): x (16384, 256) f32, params (1000, 2060) f32 -> out (16384, 1000) f32.
#
# Sharding: data-parallel over batch N across 8 cores (N_shard = 2048); params
# replicated. Per-core layout is [ck, n] (c,k merged -> 4000, padded to 4096).
# W is pre-scaled by 1/2pi on host so u arrives in "turns":
#   u1 = W1blk.T @ Xshard          (PE fp16, contraction d=256 in 2 chunks, psum f32)
#   m1 = wrap(u1 + b1') in [-.5,.5] (custom DVE op: magic-number round, 1 pass)
#   q1 = sin(2pi * m1)              (ACT Sin via free scale, fp16 out)
#   prod = q1 * q2                  (GPSIMD, fp32r out)
#   outT[cblk] += A_j.T @ prod      (PE fp32r, reduction over k with a-coeffs)
# Host: transpose/pad/scale/cast prep of x and params; final transpose of outT.

D = 256
C = 1000
K = 4
CK = C * K          # 4000
CKP = 4096          # padded
NCORES = 8
N = 16384
NS = N // NCORES    # 2048 per core
NH = 1024           # n-span per step (2 psum banks)
TWO_PI = float(2 * np.pi)
MAGIC = 12582912.0  # 1.5 * 2**23: fp32 RNE round-to-int trick
_CACHE = {}


def _enable_ldw_opt():
    """compile_bir_kernel hardcodes --enable-ldw-opt=false; flip it so walrus
    elides/overlaps repeated LDWEIGHTS (verified by the rel-err check)."""
    from concourse import bass_utils as bu

    if getattr(bu, "_bisine_ldw_patch", False):
        return
    orig = bu.run_command

    def patched(cmd, **kw):
        if isinstance(cmd, list):
            cmd = [
                "--enable-ldw-opt=true" if c == "--enable-ldw-opt=false" else c
                for c in cmd
            ]
        return orig(cmd, **kw)

    bu.run_command = patched
    bu._bisine_ldw_patch = True


def _register_wrap_op():
    """out = y - round(y) with y = in0 + s0 (per-partition bias), via the
    fp32 magic-number trick: k = (y + MAGIC) - MAGIC. Exact for |y| < 2^21."""
    import re

    from concourse import dve_ops as DV
    from concourse.dve_spec import C0, C1, Spec, Src0

    for o in DV.OPS:
        if o.name == "BISINE_WRAP":
            return o

    def _ref(in0, in1, s0, s1, imm2):
        y = (np.asarray(in0, np.float32) + np.asarray(s0, np.float32)).astype(
            np.float32
        )
        t = (y + np.float32(s1)).astype(np.float32)
        k = (t - np.float32(s1)).astype(np.float32)
        return (y - k).astype(np.float32)

    y = Src0 + C0
    k = (y + C1) - C1
    op = DV.DveOp("BISINE_WRAP", Spec(body=y - k, reference=_ref), subdim=False, uops_sha={})
    DV.OPS.append(op)
    DV.CUSTOM_DVE_SPECS[op.name] = op.spec
    DV._SUB_OPCODE_FOR_NAME[op.name] = DV._CUSTOM_DVE_ROW_BASE + len(DV.OPS) - 1
    for ver in ("v3", "v4"):
        try:
            op.compile(ver)
        except ValueError as e:
            m = re.findall(r'="([0-9a-f]+)"', str(e))
            assert m, e
            op.uops_sha[ver] = m[-1]
            op.compile(ver)
    return op


def _build_nc():
    import concourse.bacc as bacc
    import concourse.mybir as mybir
    import concourse.tile as tile

    SIN = mybir.ActivationFunctionType.Sin
    ALU = mybir.AluOpType
    BF16 = mybir.dt.bfloat16
    F16 = mybir.dt.float16
    F32 = mybir.dt.float32
    F32R = mybir.dt.float32r

    wrap_op = _register_wrap_op()
    nc = bacc.Bacc("TRN2", target_bir_lowering=False, debug=False)

    xt_d = nc.dram_tensor("xt", [D, NS], F16, kind="ExternalInput")
    w1_d = nc.dram_tensor("w1t", [D, CKP], F16, kind="ExternalInput")
    w2_d = nc.dram_tensor("w2t", [D, CKP], F16, kind="ExternalInput")
    a_d = nc.dram_tensor("acoef", [CKP, 128], F16, kind="ExternalInput")
    b1_d = nc.dram_tensor("b1v", [CKP, 1], F32, kind="ExternalInput")
    b2_d = nc.dram_tensor("b2v", [CKP, 1], F32, kind="ExternalInput")
    out_d = nc.dram_tensor("outT", [CKP // 4, NS], F32, kind="ExternalOutput")

    NJ = CKP // 128  # 32 ck-blocks
    NCB = CKP // 512  # 8 c-blocks (128 c each)

    with tile.TileContext(nc) as tc:
        with (
            tc.tile_pool(name="const", bufs=1) as cp,
            tc.tile_pool(name="work", bufs=3) as wp,
            tc.tile_pool(name="u1p", bufs=1, space="PSUM") as u1p,
            tc.tile_pool(name="u2p", bufs=1, space="PSUM") as u2p,
            tc.tile_pool(name="op", bufs=2, space="PSUM") as op,
        ):
            xt = cp.tile([128, 2, NS], F16, tag="xt")
            w1t = cp.tile([128, 2, CKP], F16, tag="w1t")
            w2t = cp.tile([128, 2, CKP], F16, tag="w2t")
            at = cp.tile([128, NJ, 128], F16, tag="at")
            b1c = cp.tile([128, NJ], F32, tag="b1c")
            b2c = cp.tile([128, NJ], F32, tag="b2c")
            zero = cp.tile([128, 1], F32, tag="zero")

            # Split DMAs so c-block 0's operands land first and compute
            # starts within a few us; the rest streams in behind it.
            w1_r = w1_d.ap().rearrange("(c p) k -> p c k", p=128)
            w2_r = w2_d.ap().rearrange("(c p) k -> p c k", p=128)
            at_r = a_d.ap().rearrange("(j p) m -> p j m", p=128)
            for cb in range(NCB):
                cs = slice(512 * cb, 512 * (cb + 1))
                for di in range(2):
                    nc.sync.dma_start(w1t[:, di, cs], w1_r[:, di, cs])
                    nc.sync.dma_start(w2t[:, di, cs], w2_r[:, di, cs])
                nc.sync.dma_start(
                    at[:, 4 * cb : 4 * (cb + 1), :], at_r[:, 4 * cb : 4 * (cb + 1), :]
                )
                if cb == 0:
                    nc.sync.dma_start(b1c[:], b1_d.ap().rearrange("(j p) o -> p (j o)", p=128))
                    nc.sync.dma_start(b2c[:], b2_d.ap().rearrange("(j p) o -> p (j o)", p=128))
                    xt_r = xt_d.ap().rearrange("(c p) n -> p c n", p=128)
                    for di in range(2):
                        nc.sync.dma_start(xt[:, di, :], xt_r[:, di, :])
            nc.gpsimd.memset(zero[:], 0.0)
            # sin argument = SCALE*m with |m| <= 0.5; SCALE one ulp under 2pi
            # keeps it strictly inside the ACT Sin [-pi, pi] domain.
            SCALE = float(np.nextafter(np.float32(TWO_PI), np.float32(0.0)))

            for cb in range(NCB):
                for nh in range(NS // NH):
                    o_ps = op.tile([128, NH], F32, tag="o_ps")
                    for jj in range(4):
                        j = 4 * cb + jj
                        u1 = u1p.tile([128, NH], F32, tag="u1")
                        u2 = u2p.tile([128, NH], F32, tag="u2")
                        jc = slice(128 * j, 128 * (j + 1))
                        # (w, d) outer / h inner: consecutive matmul pairs
                        # share lhsT so walrus ldw-opt elides the reload.
                        for u, wt in ((u1, w1t), (u2, w2t)):
                            for di in range(2):
                                for h in range(NH // 512):
                                    ncol = nh * NH + h * 512
                                    nc.tensor.matmul(
                                        u[:, h * 512 : (h + 1) * 512],
                                        wt[:, di, jc],
                                        xt[:, di, ncol : ncol + 512],
                                        start=(di == 0),
                                        stop=(di == 1),
                                    )
                        m12 = wp.tile([128, 2, NH], F32, tag="m12")
                        q12 = wp.tile([128, 2, NH], F16, tag="q12")
                        prod = wp.tile([128, NH], F16, tag="prod")
                        nc.vector._custom_dve(
                            wrap_op, out=m12[:, 0, :], in0=u1[:], s0=b1c[:, j : j + 1], s1=MAGIC
                        )
                        nc.vector._custom_dve(
                            wrap_op, out=m12[:, 1, :], in0=u2[:], s0=b2c[:, j : j + 1], s1=MAGIC
                        )
                        nc.scalar.activation(q12[:], m12[:], SIN, bias=zero[:], scale=SCALE)
                        nc.gpsimd.tensor_mul(prod[:], q12[:, 0, :], q12[:, 1, :])
                        for h in range(NH // 512):
                            c0, c1 = h * 512, (h + 1) * 512
                            nc.tensor.matmul(
                                o_ps[:, c0:c1],
                                at[:, j, :],
                                prod[:, c0:c1],
                                start=(jj == 0),
                                stop=(jj == 3),
                            )
                    o_sb = wp.tile([128, NH], F32, tag="o_sb")
                    nc.scalar.copy(o_sb[:], o_ps[:])
                    nc.sync.dma_start(
                        out_d.ap()[128 * cb : 128 * (cb + 1), nh * NH : (nh + 1) * NH],
                        o_sb[:],
                    )

    nc.compile()
    return nc


def _prep(x, params):
    p = np.asarray(params, dtype=np.float32).reshape(C, K, 2 * D + 3)
    a = np.ascontiguousarray(p[:, :, 0]).reshape(CK)
    w1 = np.ascontiguousarray(p[:, :, 1 : 1 + D]).reshape(CK, D)
    b1 = np.ascontiguousarray(p[:, :, 1 + D]).reshape(CK)
    w2 = np.ascontiguousarray(p[:, :, 2 + D : 2 + 2 * D]).reshape(CK, D)
    b2 = np.ascontiguousarray(p[:, :, 2 + 2 * D]).reshape(CK)

    w1p = np.zeros((CKP, D), np.float32)
    w2p = np.zeros((CKP, D), np.float32)
    w1p[:CK] = w1
    w2p[:CK] = w2
    inv2pi = np.float32(1.0 / TWO_PI)
    w1t = np.ascontiguousarray(w1p.T * inv2pi).astype(np.float16)
    w2t = np.ascontiguousarray(w2p.T * inv2pi).astype(np.float16)

    b1v = np.zeros((CKP, 1), np.float32)
    b2v = np.zeros((CKP, 1), np.float32)
    b1v[:CK, 0] = b1 * inv2pi
    b2v[:CK, 0] = b2 * inv2pi

    ap = np.zeros(CKP, np.float32)
    ap[:CK] = a
    # acoef[j*128+p, m] = ap[j*128+p] iff m == 32*(j%4) + p//4
    jj = np.arange(CKP) // 128
    pp = np.arange(CKP) % 128
    mcol = 32 * (jj % 4) + pp // 4
    acoef = np.zeros((CKP, 128), np.float32)
    acoef[np.arange(CKP), mcol] = ap
    acoef = acoef.astype(np.float16)

    xt = np.ascontiguousarray(np.asarray(x, dtype=np.float32).T).astype(np.float16)  # [D, N]
    return xt, w1t, w2t, acoef, b1v, b2v


def kernel(x, params):
    from concourse import bass_utils

    if "nc" not in _CACHE:
        _CACHE["nc"] = _build_nc()
    nc = _CACHE["nc"]

    xt, w1t, w2t, acoef, b1v, b2v = _prep(x, params)
    in_maps = []
    for cid in range(NCORES):
        in_maps.append(
            {
                "xt": np.ascontiguousarray(xt[:, cid * NS : (cid + 1) * NS]),
                "w1t": w1t,
                "w2t": w2t,
                "acoef": acoef,
                "b1v": b1v,
                "b2v": b2v,
            }
        )
    res = bass_utils.run_bass_kernel_spmd(nc, in_maps, core_ids=list(range(NCORES)))
    outs = [res.results[c]["outT"] for c in range(NCORES)]
    out_t = np.concatenate(outs, axis=1)  # [1024, 16384]
    return np.ascontiguousarray(out_t[:C].T)


# revision 7
# speedup vs baseline: 1.1950x; 1.1213x over previous
import sys

sys.path.insert(0, "/opt/trn_rl_repo")
import numpy as np

# nn_BisineNetwork: out[n,c] = sum_k a[c,k] * sin(x@w1[c,k]+b1[c,k]) * sin(x@w2[c,k]+b2[c,k])
# Shapes (hardcoded): x (16384, 256) f32, params (1000, 2060) f32 -> out (16384, 1000) f32.
#
# Sharding: data-parallel over batch N across 8 cores (N_shard = 2048); params
# replicated. Per-core layout is [ck, n] (c,k merged -> 4000, padded to 4096).
# W is pre-scaled by 1/2pi on host so u arrives in "turns":
#   u1 = W1blk.T @ Xshard          (PE fp16, contraction d=256 in 2 chunks, psum f32)
#   m1 = wrap(u1 + b1') in [-.5,.5] (custom DVE op: magic-number round, 1 pass)
#   q1 = sin(2pi * m1)              (ACT Sin via free scale, fp16 out)
#   prod = q1 * q2                  (GPSIMD, fp32r out)
#   outT[cblk] += A_j.T @ prod      (PE fp32r, reduction over k with a-coeffs)
# Host: transpose/pad/scale/cast prep of x and params; final transpose of outT.

D = 256
C = 1000
K = 4
CK = C * K          # 4000
CKP = 4096          # padded
NCORES = 8
N = 16384
NS = N // NCORES    # 2048 per core
NH = 1024           # n-span per step (2 psum banks)
TWO_PI = float(2 * np.pi)
MAGIC = 12582912.0  # 1.5 * 2**23: fp32 RNE round-to-int trick
_CACHE = {}


def _enable_ldw_opt():
    """compile_bir_kernel hardcodes --enable-ldw-opt=false; flip it so walrus
    elides/overlaps repeated LDWEIGHTS (verified by the rel-err check)."""
    from concourse import bass_utils as bu

    if getattr(bu, "_bisine_ldw_patch", False):
        return
    orig = bu.run_command

    def patched(cmd, **kw):
        if isinstance(cmd, list):
            cmd = [
                "--enable-ldw-opt=true" if c == "--enable-ldw-opt=false" else c
                for c in cmd
            ]
        return orig(cmd, **kw)

    bu.run_command = patched
    bu._bisine_ldw_patch = True


def _dedupe_ldweights(nc, mybir):
    """Drop PE Ldweights that reload the exact weights already resident
    (no waits/updates attached), so same-weight matmuls pipeline back to
    back instead of paying a reload + drain per matmul."""
    removed = 0
    for blk in nc.main_func.blocks:
        last_key = None
        to_remove = []
        for inst in blk.instructions:
            if isinstance(inst, mybir.InstLdweights):
                key = (
                    str(inst.ins),
                    str(inst.tile_position),
                    str(inst.perf_mode),
                    str(inst.is_transpose),
                )
                si = inst.sync_info
                clean = si is None or (len(si.on_wait) == 0 and len(si.on_update) == 0)
                if key == last_key and clean:
                    to_remove.append(inst)
                else:
                    last_key = key
            elif isinstance(inst, mybir.InstMatmult):
                pass
            elif getattr(inst, "engine", None) is not None and str(
                getattr(inst, "engine", "")
            ).endswith("PE"):
                last_key = None
        for inst in to_remove:
            blk.instructions.remove(inst)
            removed += 1
    return removed


def _register_wrap_op():
    """out = y - round(y) with y = in0 + s0 (per-partition bias), via the
    fp32 magic-number trick: k = (y + MAGIC) - MAGIC. Exact for |y| < 2^21."""
    import re

    from concourse import dve_ops as DV
    from concourse.dve_spec import C0, C1, Spec, Src0

    for o in DV.OPS:
        if o.name == "BISINE_WRAP":
            return o

    def _ref(in0, in1, s0, s1, imm2):
        y = (np.asarray(in0, np.float32) + np.asarray(s0, np.float32)).astype(
            np.float32
        )
        t = (y + np.float32(s1)).astype(np.float32)
        k = (t - np.float32(s1)).astype(np.float32)
        return (y - k).astype(np.float32)

    y = Src0 + C0
    k = (y + C1) - C1
    op = DV.DveOp("BISINE_WRAP", Spec(body=y - k, reference=_ref), subdim=False, uops_sha={})
    DV.OPS.append(op)
    DV.CUSTOM_DVE_SPECS[op.name] = op.spec
    DV._SUB_OPCODE_FOR_NAME[op.name] = DV._CUSTOM_DVE_ROW_BASE + len(DV.OPS) - 1
    for ver in ("v3", "v4"):
        try:
            op.compile(ver)
        except ValueError as e:
            m = re.findall(r'="([0-9a-f]+)"', str(e))
            assert m, e
            op.uops_sha[ver] = m[-1]
            op.compile(ver)
    return op


def _build_nc():
    import concourse.bacc as bacc
    import concourse.mybir as mybir
    import concourse.tile as tile

    SIN = mybir.ActivationFunctionType.Sin
    ALU = mybir.AluOpType
    BF16 = mybir.dt.bfloat16
    F16 = mybir.dt.float16
    F32 = mybir.dt.float32
    F32R = mybir.dt.float32r

    wrap_op = _register_wrap_op()
    nc = bacc.Bacc("TRN2", target_bir_lowering=False, debug=False)

    xt_d = nc.dram_tensor("xt", [D, NS], F16, kind="ExternalInput")
    w1_d = nc.dram_tensor("w1t", [D, CKP], F16, kind="ExternalInput")
    w2_d = nc.dram_tensor("w2t", [D, CKP], F16, kind="ExternalInput")
    a_d = nc.dram_tensor("acoef", [CKP, 32], F16, kind="ExternalInput")
    b1_d = nc.dram_tensor("b1v", [CKP, 1], F32, kind="ExternalInput")
    b2_d = nc.dram_tensor("b2v", [CKP, 1], F32, kind="ExternalInput")
    out_d = nc.dram_tensor("outT", [CKP // 4, NS], F32, kind="ExternalOutput")

    NJ = CKP // 128  # 32 ck-blocks
    NCB = CKP // 512  # 8 c-blocks (128 c each)

    with tile.TileContext(nc) as tc:
        with (
            tc.tile_pool(name="const", bufs=1) as cp,
            tc.tile_pool(name="work", bufs=3) as wp,
            tc.tile_pool(name="prodp", bufs=4) as pp_pool,
            tc.tile_pool(name="u1p", bufs=1, space="PSUM") as u1p,
            tc.tile_pool(name="u2p", bufs=1, space="PSUM") as u2p,
            tc.tile_pool(name="op", bufs=2, space="PSUM") as op,
        ):
            xt = cp.tile([128, 2, NS], F16, tag="xt")
            w1t = cp.tile([128, 2, CKP], F16, tag="w1t")
            w2t = cp.tile([128, 2, CKP], F16, tag="w2t")
            at = cp.tile([128, NJ, 32], F16, tag="at")
            b1c = cp.tile([128, NJ], F32, tag="b1c")
            b2c = cp.tile([128, NJ], F32, tag="b2c")
            zero = cp.tile([128, 1], F32, tag="zero")

            # Split DMAs so c-block 0's operands land first and compute
            # starts within a few us; the rest streams in behind it.
            w1_r = w1_d.ap().rearrange("(c p) k -> p c k", p=128)
            w2_r = w2_d.ap().rearrange("(c p) k -> p c k", p=128)
            at_r = a_d.ap().rearrange("(j p) m -> p j m", p=128)
            for cb in range(NCB):
                cs = slice(512 * cb, 512 * (cb + 1))
                for di in range(2):
                    nc.sync.dma_start(w1t[:, di, cs], w1_r[:, di, cs])
                    nc.sync.dma_start(w2t[:, di, cs], w2_r[:, di, cs])
                nc.sync.dma_start(
                    at[:, 4 * cb : 4 * (cb + 1), :], at_r[:, 4 * cb : 4 * (cb + 1), :]
                )
                if cb == 0:
                    nc.sync.dma_start(b1c[:], b1_d.ap().rearrange("(j p) o -> p (j o)", p=128))
                    nc.sync.dma_start(b2c[:], b2_d.ap().rearrange("(j p) o -> p (j o)", p=128))
                    xt_r = xt_d.ap().rearrange("(c p) n -> p c n", p=128)
                    for di in range(2):
                        nc.sync.dma_start(xt[:, di, :], xt_r[:, di, :])
            nc.gpsimd.memset(zero[:], 0.0)
            # sin argument = SCALE*m with |m| <= 0.5; SCALE one ulp under 2pi
            # keeps it strictly inside the ACT Sin [-pi, pi] domain.
            SCALE = float(np.nextafter(np.float32(TWO_PI), np.float32(0.0)))

            # Reduction matmuls are deferred DELAY steps so the PE never
            # waits on the wrap -> sin -> prod chain of the current step.
            DELAY = 2
            pending = []
            ostate = {}

            def flush_one():
                cb, nh, jj, j, prod = pending.pop(0)
                if jj == 0:
                    ostate[(cb, nh)] = op.tile([128, NH], F32, tag="o_ps", name="o_ps")
                o_ps = ostate[(cb, nh)]
                po = 32 * jj
                for h in range(NH // 512):
                    c0, c1 = h * 512, (h + 1) * 512
                    nc.tensor.matmul(
                        o_ps[po : po + 32, c0:c1],
                        at[:, j, :],
                        prod[:, c0:c1],
                        start=True,
                        stop=True,
                        tile_position=(0, po),
                    )
                if jj == 3:
                    o_sb = wp.tile([128, NH], F32, tag="o_sb")
                    nc.scalar.copy(o_sb[:], o_ps[:])
                    nc.sync.dma_start(
                        out_d.ap()[
                            128 * cb : 128 * (cb + 1), nh * NH : (nh + 1) * NH
                        ],
                        o_sb[:],
                    )
                    del ostate[(cb, nh)]

            for cb in range(NCB):
                for nh in range(NS // NH):
                    for jj in range(4):
                        j = 4 * cb + jj
                        u1 = u1p.tile([128, NH], F32, tag="u1")
                        u2 = u2p.tile([128, NH], F32, tag="u2")
                        jc = slice(128 * j, 128 * (j + 1))
                        # (w, d) outer / h inner: consecutive matmuls share
                        # lhsT; _dedupe_ldweights drops the reload between
                        # them so the pair pipelines at stream rate.
                        for u, wt in ((u1, w1t), (u2, w2t)):
                            for di in range(2):
                                for h in range(NH // 512):
                                    ncol = nh * NH + h * 512
                                    nc.tensor.matmul(
                                        u[:, h * 512 : (h + 1) * 512],
                                        wt[:, di, jc],
                                        xt[:, di, ncol : ncol + 512],
                                        start=(di == 0),
                                        stop=(di == 1),
                                    )
                        m12 = wp.tile([128, 2, NH], F32, tag="m12")
                        q12 = wp.tile([128, 2, NH], F16, tag="q12")
                        prod = pp_pool.tile([128, NH], F16, tag="prod")
                        nc.vector._custom_dve(
                            wrap_op, out=m12[:, 0, :], in0=u1[:], s0=b1c[:, j : j + 1], s1=MAGIC
                        )
                        nc.vector._custom_dve(
                            wrap_op, out=m12[:, 1, :], in0=u2[:], s0=b2c[:, j : j + 1], s1=MAGIC
                        )
                        nc.scalar.activation(q12[:], m12[:], SIN, bias=zero[:], scale=SCALE)
                        nc.gpsimd.tensor_mul(prod[:], q12[:, 0, :], q12[:, 1, :])
                        pending.append((cb, nh, jj, j, prod))
                        if len(pending) > DELAY:
                            flush_one()
            while pending:
                flush_one()

    _dedupe_ldweights(nc, mybir)
    nc.compile()
    return nc


def _prep(x, params):
    p = np.asarray(params, dtype=np.float32).reshape(C, K, 2 * D + 3)
    a = np.ascontiguousarray(p[:, :, 0]).reshape(CK)
    w1 = np.ascontiguousarray(p[:, :, 1 : 1 + D]).reshape(CK, D)
    b1 = np.ascontiguousarray(p[:, :, 1 + D]).reshape(CK)
    w2 = np.ascontiguousarray(p[:, :, 2 + D : 2 + 2 * D]).reshape(CK, D)
    b2 = np.ascontiguousarray(p[:, :, 2 + 2 * D]).reshape(CK)

    w1p = np.zeros((CKP, D), np.float32)
    w2p = np.zeros((CKP, D), np.float32)
    w1p[:CK] = w1
    w2p[:CK] = w2
    inv2pi = np.float32(1.0 / TWO_PI)
    w1t = np.ascontiguousarray(w1p.T * inv2pi).astype(np.float16)
    w2t = np.ascontiguousarray(w2p.T * inv2pi).astype(np.float16)

    b1v = np.zeros((CKP, 1), np.float32)
    b2v = np.zeros((CKP, 1), np.float32)
    b1v[:CK, 0] = b1 * inv2pi
    b2v[:CK, 0] = b2 * inv2pi

    ap = np.zeros(CKP, np.float32)
    ap[:CK] = a
    # acoef[row, m] = ap[row] iff m == (row % 128)//4; the 32-wide output
    # lands at psum partition offset 32*(j%4) via matmul tile_position.
    pp = np.arange(CKP) % 128
    acoef = np.zeros((CKP, 32), np.float32)
    acoef[np.arange(CKP), pp // 4] = ap
    acoef = acoef.astype(np.float16)

    xt = np.ascontiguousarray(np.asarray(x, dtype=np.float32).T).astype(np.float16)  # [D, N]
    return xt, w1t, w2t, acoef, b1v, b2v


def kernel(x, params):
    from concourse import bass_utils

    if "nc" not in _CACHE:
        _CACHE["nc"] = _build_nc()
    nc = _CACHE["nc"]

    xt, w1t, w2t, acoef, b1v, b2v = _prep(x, params)
    in_maps = []
    for cid in range(NCORES):
        in_maps.append(
            {
                "xt": np.ascontiguousarray(xt[:, cid * NS : (cid + 1) * NS]),
                "w1t": w1t,
                "w2t": w2t,
                "acoef": acoef,
                "b1v": b1v,
                "b2v": b2v,
            }
        )
    res = bass_utils.run_bass_kernel_spmd(nc, in_maps, core_ids=list(range(NCORES)))
    outs = [res.results[c]["outT"] for c in range(NCORES)]
    out_t = np.concatenate(outs, axis=1)  # [1024, 16384]
    return np.ascontiguousarray(out_t[:C].T)


# revision 8
# speedup vs baseline: 1.2204x; 1.0212x over previous
import sys

sys.path.insert(0, "/opt/trn_rl_repo")
import numpy as np

# nn_BisineNetwork: out[n,c] = sum_k a[c,k] * sin(x@w1[c,k]+b1[c,k]) * sin(x@w2[c,k]+b2[c,k])
# Shapes (hardcoded): x (16384, 256) f32, params (1000, 2060) f32 -> out (16384, 1000) f32.
#
# Sharding: data-parallel over batch N across 8 cores (N_shard = 2048); params
# replicated. Per-core layout is [ck, n] (c,k merged -> 4000, padded to 4096).
# W is pre-scaled by 1/2pi on host so u arrives in "turns":
#   u1 = W1blk.T @ Xshard          (PE fp16, contraction d=256 in 2 chunks, psum f32)
#   m1 = wrap(u1 + b1') in [-.5,.5] (custom DVE op: magic-number round, 1 pass)
#   q1 = sin(2pi * m1)              (ACT Sin via free scale, fp16 out)
#   prod = q1 * q2                  (GPSIMD, fp32r out)
#   outT[cblk] += A_j.T @ prod      (PE fp32r, reduction over k with a-coeffs)
# Host: transpose/pad/scale/cast prep of x and params; final transpose of outT.

D = 256
C = 1000
K = 4
CK = C * K          # 4000
CKP = 4096          # padded
NCORES = 8
N = 16384
NS = N // NCORES    # 2048 per core
NH = 1024           # n-span per step (2 psum banks)
TWO_PI = float(2 * np.pi)
MAGIC = 12582912.0  # 1.5 * 2**23: fp32 RNE round-to-int trick
_CACHE = {}


def _enable_ldw_opt():
    """compile_bir_kernel hardcodes --enable-ldw-opt=false; flip it so walrus
    elides/overlaps repeated LDWEIGHTS (verified by the rel-err check)."""
    from concourse import bass_utils as bu

    if getattr(bu, "_bisine_ldw_patch", False):
        return
    orig = bu.run_command

    def patched(cmd, **kw):
        if isinstance(cmd, list):
            cmd = [
                "--enable-ldw-opt=true" if c == "--enable-ldw-opt=false" else c
                for c in cmd
            ]
        return orig(cmd, **kw)

    bu.run_command = patched
    bu._bisine_ldw_patch = True


def _dedupe_ldweights(nc, mybir):
    """Drop PE Ldweights that reload the exact weights already resident
    (no waits/updates attached), so same-weight matmuls pipeline back to
    back instead of paying a reload + drain per matmul."""
    removed = 0
    for blk in nc.main_func.blocks:
        last_key = None
        to_remove = []
        for inst in blk.instructions:
            if isinstance(inst, mybir.InstLdweights):
                key = (
                    str(inst.ins),
                    str(inst.tile_position),
                    str(inst.perf_mode),
                    str(inst.is_transpose),
                )
                si = inst.sync_info
                clean = si is None or (len(si.on_wait) == 0 and len(si.on_update) == 0)
                if key == last_key and clean:
                    to_remove.append(inst)
                else:
                    last_key = key
            elif isinstance(inst, mybir.InstMatmult):
                pass
            elif getattr(inst, "engine", None) is not None and str(
                getattr(inst, "engine", "")
            ).endswith("PE"):
                last_key = None
        for inst in to_remove:
            blk.instructions.remove(inst)
            removed += 1
    return removed


def _register_wrap_op():
    """out = y - round(y) with y = in0 + s0 (per-partition bias), via the
    fp32 magic-number trick: k = (y + MAGIC) - MAGIC. Exact for |y| < 2^21."""
    import re

    from concourse import dve_ops as DV
    from concourse.dve_spec import C0, C1, Spec, Src0

    for o in DV.OPS:
        if o.name == "BISINE_WRAP":
            return o

    def _ref(in0, in1, s0, s1, imm2):
        y = (np.asarray(in0, np.float32) + np.asarray(s0, np.float32)).astype(
            np.float32
        )
        t = (y + np.float32(s1)).astype(np.float32)
        k = (t - np.float32(s1)).astype(np.float32)
        return (y - k).astype(np.float32)

    y = Src0 + C0
    k = (y + C1) - C1
    op = DV.DveOp("BISINE_WRAP", Spec(body=y - k, reference=_ref), subdim=False, uops_sha={})
    DV.OPS.append(op)
    DV.CUSTOM_DVE_SPECS[op.name] = op.spec
    DV._SUB_OPCODE_FOR_NAME[op.name] = DV._CUSTOM_DVE_ROW_BASE + len(DV.OPS) - 1
    for ver in ("v3", "v4"):
        try:
            op.compile(ver)
        except ValueError as e:
            m = re.findall(r'="([0-9a-f]+)"', str(e))
            assert m, e
            op.uops_sha[ver] = m[-1]
            op.compile(ver)
    return op


def _build_nc():
    import concourse.bacc as bacc
    import concourse.mybir as mybir
    import concourse.tile as tile

    SIN = mybir.ActivationFunctionType.Sin
    ALU = mybir.AluOpType
    BF16 = mybir.dt.bfloat16
    F16 = mybir.dt.float16
    F32 = mybir.dt.float32
    F32R = mybir.dt.float32r

    wrap_op = _register_wrap_op()
    nc = bacc.Bacc("TRN2", target_bir_lowering=False, debug=False)

    xt_d = nc.dram_tensor("xt", [D, NS], F16, kind="ExternalInput")
    w1_d = nc.dram_tensor("w1t", [D, CKP], F16, kind="ExternalInput")
    w2_d = nc.dram_tensor("w2t", [D, CKP], F16, kind="ExternalInput")
    a_d = nc.dram_tensor("acoef", [CKP, 32], F16, kind="ExternalInput")
    b1_d = nc.dram_tensor("b1v", [CKP, 1], F32, kind="ExternalInput")
    b2_d = nc.dram_tensor("b2v", [CKP, 1], F32, kind="ExternalInput")
    out_d = nc.dram_tensor("outT", [CKP // 4, NS], F32, kind="ExternalOutput")

    NJ = CKP // 128  # 32 ck-blocks
    NCB = CKP // 512  # 8 c-blocks (128 c each)

    with tile.TileContext(nc) as tc:
        with (
            tc.tile_pool(name="const", bufs=1) as cp,
            tc.tile_pool(name="work", bufs=3) as wp,
            tc.tile_pool(name="prodp", bufs=4) as pp_pool,
            tc.tile_pool(name="u1p", bufs=1, space="PSUM") as u1p,
            tc.tile_pool(name="u2p", bufs=1, space="PSUM") as u2p,
            tc.tile_pool(name="op", bufs=2, space="PSUM") as op,
        ):
            xt = cp.tile([128, 2, NS], F16, tag="xt")
            w1t = cp.tile([128, 2, CKP], F16, tag="w1t")
            w2t = cp.tile([128, 2, CKP], F16, tag="w2t")
            at = cp.tile([128, NJ, 32], F16, tag="at")
            b1c = cp.tile([128, NJ], F32, tag="b1c")
            b2c = cp.tile([128, NJ], F32, tag="b2c")
            zero = cp.tile([128, 1], F32, tag="zero")

            # Split DMAs so the first step's operands land first: weights on
            # the sync queue, x on the gpsimd queue (runs in parallel).
            w1_r = w1_d.ap().rearrange("(c p) k -> p c k", p=128)
            w2_r = w2_d.ap().rearrange("(c p) k -> p c k", p=128)
            at_r = a_d.ap().rearrange("(j p) m -> p j m", p=128)
            xt_r = xt_d.ap().rearrange("(c p) n -> p c n", p=128)
            for di in range(2):
                for hh in range(2):
                    hs = slice(NH * hh, NH * (hh + 1))
                    nc.gpsimd.dma_start(xt[:, di, hs], xt_r[:, di, hs])
            for cb in range(NCB):
                jblk = slice(128 * (4 * cb), 128 * (4 * cb + 1))
                for di in range(2):
                    nc.sync.dma_start(w1t[:, di, jblk], w1_r[:, di, jblk])
                cs = slice(512 * cb, 512 * (cb + 1))
                for di in range(2):
                    nc.sync.dma_start(w2t[:, di, cs], w2_r[:, di, cs])
                # w1 rest of the c-block (j 1..3) behind w2's first block
                rest = slice(128 * (4 * cb) + 128, 512 * (cb + 1))
                for di in range(2):
                    nc.sync.dma_start(w1t[:, di, rest], w1_r[:, di, rest])
                nc.sync.dma_start(
                    at[:, 4 * cb : 4 * (cb + 1), :], at_r[:, 4 * cb : 4 * (cb + 1), :]
                )
                if cb == 0:
                    nc.sync.dma_start(b1c[:], b1_d.ap().rearrange("(j p) o -> p (j o)", p=128))
                    nc.sync.dma_start(b2c[:], b2_d.ap().rearrange("(j p) o -> p (j o)", p=128))
            nc.gpsimd.memset(zero[:], 0.0)
            # sin argument = SCALE*m with |m| <= 0.5; SCALE one ulp under 2pi
            # keeps it strictly inside the ACT Sin [-pi, pi] domain.
            SCALE = float(np.nextafter(np.float32(TWO_PI), np.float32(0.0)))

            # Reduction matmuls are deferred DELAY steps so the PE never
            # waits on the wrap -> sin -> prod chain of the current step.
            DELAY = 2
            pending = []
            ostate = {}

            def flush_one():
                cb, nh, jj, j, prod = pending.pop(0)
                if jj == 0:
                    ostate[(cb, nh)] = op.tile([128, NH], F32, tag="o_ps", name="o_ps")
                o_ps = ostate[(cb, nh)]
                po = 32 * jj
                for h in range(NH // 512):
                    c0, c1 = h * 512, (h + 1) * 512
                    nc.tensor.matmul(
                        o_ps[po : po + 32, c0:c1],
                        at[:, j, :],
                        prod[:, c0:c1],
                        start=True,
                        stop=True,
                        tile_position=(0, po),
                    )
                if jj == 3:
                    o_sb = wp.tile([128, NH], F32, tag="o_sb")
                    nc.scalar.copy(o_sb[:], o_ps[:])
                    nc.sync.dma_start(
                        out_d.ap()[
                            128 * cb : 128 * (cb + 1), nh * NH : (nh + 1) * NH
                        ],
                        o_sb[:],
                    )
                    del ostate[(cb, nh)]

            for cb in range(NCB):
                for jj in range(4):
                    for nh in range(NS // NH):
                        j = 4 * cb + jj
                        u1 = u1p.tile([128, NH], F32, tag="u1")
                        u2 = u2p.tile([128, NH], F32, tag="u2")
                        jc = slice(128 * j, 128 * (j + 1))
                        # (w, d) outer / h inner: consecutive matmuls share
                        # lhsT; _dedupe_ldweights drops the reload between
                        # them so the pair pipelines at stream rate.
                        for u, wt in ((u1, w1t), (u2, w2t)):
                            for di in range(2):
                                for h in range(NH // 512):
                                    ncol = nh * NH + h * 512
                                    nc.tensor.matmul(
                                        u[:, h * 512 : (h + 1) * 512],
                                        wt[:, di, jc],
                                        xt[:, di, ncol : ncol + 512],
                                        start=(di == 0),
                                        stop=(di == 1),
                                    )
                        m12 = wp.tile([128, 2, NH], F32, tag="m12")
                        q12 = wp.tile([128, 2, NH], F16, tag="q12")
                        prod = pp_pool.tile([128, NH], F16, tag="prod")
                        nc.vector._custom_dve(
                            wrap_op, out=m12[:, 0, :], in0=u1[:], s0=b1c[:, j : j + 1], s1=MAGIC
                        )
                        nc.vector._custom_dve(
                            wrap_op, out=m12[:, 1, :], in0=u2[:], s0=b2c[:, j : j + 1], s1=MAGIC
                        )
                        nc.scalar.activation(q12[:], m12[:], SIN, bias=zero[:], scale=SCALE)
                        nc.gpsimd.tensor_mul(prod[:], q12[:, 0, :], q12[:, 1, :])
                        pending.append((cb, nh, jj, j, prod))
                        if len(pending) > DELAY:
                            flush_one()
            while pending:
                flush_one()

    _dedupe_ldweights(nc, mybir)
    nc.compile()
    return nc


def _prep(x, params):
    p = np.asarray(params, dtype=np.float32).reshape(C, K, 2 * D + 3)
    a = np.ascontiguousarray(p[:, :, 0]).reshape(CK)
    w1 = np.ascontiguousarray(p[:, :, 1 : 1 + D]).reshape(CK, D)
    b1 = np.ascontiguousarray(p[:, :, 1 + D]).reshape(CK)
    w2 = np.ascontiguousarray(p[:, :, 2 + D : 2 + 2 * D]).reshape(CK, D)
    b2 = np.ascontiguousarray(p[:, :, 2 + 2 * D]).reshape(CK)

    w1p = np.zeros((CKP, D), np.float32)
    w2p = np.zeros((CKP, D), np.float32)
    w1p[:CK] = w1
    w2p[:CK] = w2
    inv2pi = np.float32(1.0 / TWO_PI)
    w1t = np.ascontiguousarray(w1p.T * inv2pi).astype(np.float16)
    w2t = np.ascontiguousarray(w2p.T * inv2pi).astype(np.float16)

    b1v = np.zeros((CKP, 1), np.float32)
    b2v = np.zeros((CKP, 1), np.float32)
    b1v[:CK, 0] = b1 * inv2pi
    b2v[:CK, 0] = b2 * inv2pi

    ap = np.zeros(CKP, np.float32)
    ap[:CK] = a
    # acoef[row, m] = ap[row] iff m == (row % 128)//4; the 32-wide output
    # lands at psum partition offset 32*(j%4) via matmul tile_position.
    pp = np.arange(CKP) % 128
    acoef = np.zeros((CKP, 32), np.float32)
    acoef[np.arange(CKP), pp // 4] = ap
    acoef = acoef.astype(np.float16)

    xt = np.ascontiguousarray(np.asarray(x, dtype=np.float32).T).astype(np.float16)  # [D, N]
    return xt, w1t, w2t, acoef, b1v, b2v


def kernel(x, params):
    from concourse import bass_utils

    if "nc" not in _CACHE:
        _CACHE["nc"] = _build_nc()
    nc = _CACHE["nc"]

    xt, w1t, w2t, acoef, b1v, b2v = _prep(x, params)
    in_maps = []
    for cid in range(NCORES):
        in_maps.append(
            {
                "xt": np.ascontiguousarray(xt[:, cid * NS : (cid + 1) * NS]),
                "w1t": w1t,
                "w2t": w2t,
                "acoef": acoef,
                "b1v": b1v,
                "b2v": b2v,
            }
        )
    res = bass_utils.run_bass_kernel_spmd(nc, in_maps, core_ids=list(range(NCORES)))
    outs = [res.results[c]["outT"] for c in range(NCORES)]
    out_t = np.concatenate(outs, axis=1)  # [1024, 16384]
    return np.ascontiguousarray(out_t[:C].T)
